# revision 9
# baseline (speedup 1.0000x reference)
import sys

if "/opt/trn_rl_repo" not in sys.path:
    sys.path.insert(0, "/opt/trn_rl_repo")

import numpy as np
import concourse.bass as bass
import concourse.bacc as bacc
import concourse.mybir as mybir
import concourse.tile as tile
from concourse import bass_utils
import ml_dtypes

jnp_bf16 = ml_dtypes.bfloat16

B, T, I, H, C = 512, 1024, 64, 128, 10
NCORES = 8
BL = B // NCORES          # batch per core (64)
HB = BL // 2              # half-batch (32)
GS = 4                    # steps per psum group
CH = 128                  # timesteps per x DMA chunk
FP32 = mybir.dt.float32
BF16 = mybir.dt.bfloat16
ALU = mybir.AluOpType

_cache = {}

# Two phase-shifted half-batches (A = batch 0:32, B = 32:64), one psum
# group tile [128, 4*GS*32] (one bank) per half per group, 4 buffers.
#
# Per half, a strided W tile [128, 384] bf16 holds the per-step state in
# batch-interleaved triples (psum/ACT gate order is (g, i, f, o)):
#   col 3b+0 in [  0: 96): s2g_b = sig(2*g_pre)   (ACT out, stride 3)
#   col 3b+1 in [  0: 96): c~_b  = c/2 + 1/2      (gpsimd stt2 out)
#   col 3b+0 in [ 96:192): si_b                   (ACT out)
#   col 3b+0 in [192:288): sf_b                   (ACT out)
#   col 286+3b           : 0.0  \ scan multiplier pattern
#   col 287+3b           : 1.0  / (constants, memset once)
#   col 288+3b in [288:384): so_b                 (ACT out)
# XS tile [128, 96]: col 3b=0 (const), 3b+1 = q_b, 3b+2 = r_b  (stt1 out)
# OUT tile [128, 96]: scan output, hh_b = h_b/2 at col 3b+2
#
# Per half-step:
#   ACT    : [s2g|si|sf|so] = sigmoid(psum gates)        (strided dst)
#   stt1   : [q|r] = ([s2g|c~] - 0.5) * [si|sf]          (DVE)
#   scan   : state=(d0+state)*d1 over (junk,q,r)x(0,1,so) -> hh=(r+q)*so
#   stt2   : c~ = (r + 0.5) + q                          (GpSimd, off-path)
# hh feeds the next step's h-projection matmuls (Wh, Wfc pre-doubled).


def _build():
    nc = bacc.Bacc("TRN2", debug=False, num_devices=NCORES)
    xt_d = nc.dram_tensor("xt", [I + 1, T * BL], BF16, kind="ExternalInput")
    wx_d = nc.dram_tensor("wx", [I + 1, 4 * H], BF16, kind="ExternalInput")
    wh_d = nc.dram_tensor("wh", [H, 4 * H], BF16, kind="ExternalInput")
    wfc_d = nc.dram_tensor("wfc", [H, C], BF16, kind="ExternalInput")
    bfc_d = nc.dram_tensor("bfcb", [C, BL], FP32, kind="ExternalInput")
    y_d = nc.dram_tensor("y", [C, BL], FP32, kind="ExternalOutput")

    GC = GS * HB  # columns per gate per half-group (128)

    with tile.TileContext(nc) as tc:
        with (
            tc.tile_pool(name="const", bufs=1) as cpool,
            tc.tile_pool(name="xch", bufs=2) as xpool,
            tc.tile_pool(name="psA", bufs=4, space="PSUM") as ppoolA,
            tc.tile_pool(name="psB", bufs=4, space="PSUM") as ppoolB,
        ):
            wx_s = cpool.tile([I + 1, 4 * H], BF16)
            wh_s = cpool.tile([H, 4 * H], BF16)
            wfc_s = cpool.tile([H, C], BF16)
            bfc_s = cpool.tile([C, BL], FP32)
            W = [cpool.tile([128, 384], BF16, name=f"W{h}") for h in range(2)]
            XS = [cpool.tile([128, 96], BF16, name=f"X{h}") for h in range(2)]
            OUT = [cpool.tile([128, 96], BF16, name=f"O{h}") for h in range(2)]

            def tri(t, base, j):
                # strided view: [128, 32] selecting cols {base + 3b + j}
                return t[:, base : base + 96].rearrange(
                    "p (b j) -> p b j", j=3
                )[:, :, j]

            nc.sync.dma_start(wx_s[:], wx_d.ap())
            nc.sync.dma_start(wh_s[:], wh_d.ap())
            nc.sync.dma_start(wfc_s[:], wfc_d.ap())
            nc.sync.dma_start(bfc_s[:], bfc_d.ap())
            for h in range(2):
                nc.vector.memset(W[h][:], 0.0)
                nc.vector.memset(tri(W[h], 0, 1), 0.5)    # c~ = 0.5 (c=0)
                nc.vector.memset(tri(W[h], 285, 2), 1.0)  # cols 287+3b = 1.0
                nc.vector.memset(XS[h][:], 0.0)
                nc.vector.memset(OUT[h][:], 0.0)

            pools = [ppoolA, ppoolB]
            NG = T // GS

            def alloc_group():
                return [
                    pools[h].tile([128, 4 * GC], FP32, tag="ps", name=f"ps{h}")
                    for h in range(2)
                ]

            def emit_xproj_one(grp, xc, P, k):
                # one x-projection matmul (gate g of half h).  The whole
                # [128, 4*GC] half-tile is one psum bank; start=True only on
                # the bank's first matmul (gate 0) opens the accumulation
                # group, everything after accumulates.
                xcv = xc.rearrange("p (t n) -> p t n", n=BL)
                t0 = (grp % (CH // GS)) * GS
                h, g = k // 4, k % 4
                nc.tensor.matmul(
                    P[h][:, g * GC : (g + 1) * GC],
                    wx_s[:, g * H : (g + 1) * H],
                    xcv[:, t0 : t0 + GS, h * HB : (h + 1) * HB],
                    start=(g == 0),
                    stop=False,
                )

            def fetch_chunk(grp):
                ci = grp // (CH // GS)
                xc = xpool.tile([I + 1, CH * BL], BF16)
                nc.sync.dma_start(
                    xc[:], xt_d.ap()[:, ci * CH * BL : (ci + 1) * CH * BL]
                )
                return xc

            xc = fetch_chunk(0)
            P = alloc_group()
            for k in range(8):
                emit_xproj_one(0, xc, P, k)
            for grp in range(NG):
                P_next = None
                for s in range(GS):
                    if grp + 1 < NG and s == 0:
                        if (grp + 1) % (CH // GS) == 0:
                            xc = fetch_chunk(grp + 1)
                        P_next = alloc_group()
                    for h in range(2):
                        Ph = P[h]
                        for g in range(4):
                            nc.tensor.matmul(
                                Ph[:, g * GC + s * HB : g * GC + (s + 1) * HB],
                                wh_s[:, g * H : (g + 1) * H],
                                tri(OUT[h], 0, 2),
                                start=False,
                                stop=(s == GS - 1 and g == 3),
                            )
                        Wt = W[h]
                        src = Ph.rearrange("p (g s n) -> p g s n", g=4, s=GS)[
                            :, :, s, :
                        ]
                        # dst: gate g -> cols {96*g + 3b}
                        dst = Wt.rearrange("p (g b j) -> p g b j", g=4, j=3)[
                            :, :, :, 0
                        ]
                        nc.scalar.activation(
                            dst, src, mybir.ActivationFunctionType.Sigmoid
                        )
                        # stt1: [q|r] = ([s2g|c~] - 0.5) * [si|sf]
                        in0 = Wt[:, 0:96].rearrange("p (b j) -> p b j", j=3)[
                            :, :, 0:2
                        ]
                        in1 = Wt[:, 96:288].rearrange(
                            "p (j2 b j) -> p b j2 j", j2=2, j=3
                        )[:, :, :, 0]
                        out1 = XS[h].rearrange("p (b j) -> p b j", j=3)[
                            :, :, 1:3
                        ]
                        nc.vector.scalar_tensor_tensor(
                            out1, in0, 0.5, in1, ALU.subtract, ALU.mult
                        )
                        # scan: hh_b = (r_b + q_b) * so_b at out col 3b+2
                        nc.vector.tensor_tensor_scan(
                            OUT[h][:, 0:96],
                            XS[h][:, 0:96],
                            Wt[:, 286:382],
                            0.0,
                            ALU.add,
                            ALU.mult,
                        )
                        # stt2 (off critical path): c~ = (r + 0.5) + q
                        nc.vector.scalar_tensor_tensor(
                            tri(Wt, 0, 1),
                            tri(XS[h], 0, 2),
                            0.5,
                            tri(XS[h], 0, 1),
                            ALU.add,
                            ALU.add,
                        )
                    # next group's x-projection matmuls go BEHIND this
                    # step's critical h-matmuls in the in-order PE queue
                    if grp + 1 < NG:
                        emit_xproj_one(grp + 1, xc, P_next, 2 * s)
                        emit_xproj_one(grp + 1, xc, P_next, 2 * s + 1)
                if P_next is not None:
                    P = P_next

            ypt = ppoolA.tile([128, 4 * GC], FP32, tag="ps")
            yp = ypt[:C, :BL]
            for h in range(2):
                nc.tensor.matmul(
                    ypt[:C, h * HB : (h + 1) * HB],
                    wfc_s[:],
                    tri(OUT[h], 0, 2),
                    start=(h == 0),
                    stop=(h == 1),
                )
            y_s = cpool.tile([C, BL], FP32)
            nc.vector.tensor_add(y_s[:], yp, bfc_s[:])
            nc.sync.dma_start(y_d.ap(), y_s[:])

    nc.compile()
    return nc


def kernel(x, Wf, bf, Wo, bo, Wi, bi, Wg, bg, Wfc, bfc):
    if "nc" not in _cache:
        _cache["nc"] = _build()
    nc = _cache["nc"]

    # gate order (g, i, f, o); g rows pre-scaled x2 (tanh z = 2*sig(2z)-1);
    # wh doubled because the matmul consumes hh = h/2; wfc likewise doubled.
    gates = [(Wg, bg, 2.0), (Wi, bi, 1.0), (Wf, bf, 1.0), (Wo, bo, 1.0)]
    wx = np.concatenate(
        [
            s * np.concatenate([W[:, :I].T, b[None, :]], axis=0)
            for W, b, s in gates
        ],
        axis=1,
    ).astype(np.float32).astype(jnp_bf16)  # [I+1, 4H]
    wh = np.concatenate(
        [2.0 * s * W[:, I:].T for W, _, s in gates], axis=1
    ).astype(np.float32).astype(jnp_bf16)  # [H, 4H]
    wfc = np.ascontiguousarray(2.0 * Wfc.T).astype(np.float32).astype(jnp_bf16)
    bfcb = np.broadcast_to(bfc[:, None], (C, BL)).astype(np.float32).copy()

    in_maps = []
    for cidx in range(NCORES):
        xs = np.asarray(x[cidx * BL : (cidx + 1) * BL], np.float32)  # [BL,T,I]
        xt = np.ascontiguousarray(xs.transpose(2, 1, 0)).reshape(I, T * BL)
        xt = np.concatenate(
            [xt, np.ones((1, T * BL), np.float32)], axis=0
        ).astype(jnp_bf16)
        in_maps.append({"xt": xt, "wx": wx, "wh": wh, "wfc": wfc, "bfcb": bfcb})

    _cache["in_maps"] = in_maps
    res = bass_utils.run_bass_kernel_spmd(
        nc, in_maps, core_ids=list(range(NCORES))
    )
    return np.concatenate([r["y"].T for r in res.results], axis=0)
